# revision 10
# baseline (speedup 1.0000x reference)
"""Causal self-attention (B=4, T=2048, C=1024, H=16) on 8 TRN2 NeuronCores.

Sharding: batch x head-halves. Core i handles batch b=i//2 and heads
[8*(i%2), 8*(i%2)+8). Each core computes QKV projection for its slice,
causal attention for its 8 heads, and a partial output projection
(512 of 1024 contraction features). The host sums the two partials per
batch and transposes back.

All matmul operands are bf16 (fp32 PSUM accumulation): 1 cycle/row on
the PE with no small-N penalty, and half the DMA/SBUF footprint of
fp32 -- the startup weight+activation load that gates the first real
matmuls drops from 6MB to 3MB, keeping the PE out of its cold
(1.2GHz) HAM state sooner.

Attention per head works on S^T tiles: S^T[k, q] so that the AV matmul
(lhsT = V [k, d+1], rhs = P^T [k, q]) needs no transposes, with a ones
column appended to V so row 64 of the PSUM accumulator collects the
softmax denominators. The two heads of a pair run their QK^T matmuls
concurrently in the PE array (disjoint 64-row groups). exp happens on
the scalar engine PSUM->SBUF with the 1/8 scale folded in; no max
subtraction (scores are ~N(0,1), safe in fp32). Causality: (k-tile,
q-chunk) pairs above the diagonal are skipped, diagonal tiles compute
only the valid columns and get a triangular mask multiply.

Softmax normalization stays on-chip: the denominator row of o_un is
broadcast down 64 partitions with a K=1 matmul against a ones row
(lhsT [1,64]), reciprocal'd lane-parallel on the vector engine
straight out of PSUM, and multiplied into O^T. No DRAM bounce, no
partition-scatter DMAs. Each head-pair's normalization is emitted one
head-pair late so the tiny broadcast matmul never parks at the head of
the PE queue waiting for the o_un copy.

Pipelining: phase1 (projections, per 512-t-chunk), phase2 (attention,
per 512-q-chunk) and phase3 (out-proj, one chunk behind) are emitted
interleaved -- legal because causal attention for q-chunk j only needs
K/V of chunks <= j.
"""

import sys

if "/opt/trn_rl_repo" not in sys.path:
    sys.path.insert(0, "/opt/trn_rl_repo")

import numpy as np
import ml_dtypes

import concourse.bass as bass
import concourse.mybir as mybir
import concourse.tile as tile
from concourse import bacc
from concourse.bass_utils import run_bass_kernel_spmd
from concourse.masks import make_upper_triangular

B, T, C, H = 4, 2048, 1024, 16
HD = C // H  # 64
NCORES = 8
HPC = H // 2  # heads per core = 8
F = HPC * HD  # 512 features per core
CH = 512  # t/q chunk width
NCH = T // CH  # 4
NKT = T // 128  # 16 k-tiles

f32 = mybir.dt.float32
f32r = mybir.dt.float32r
bf16 = mybir.dt.bfloat16


def build_nc():
    nc = bacc.Bacc("TRN2", target_bir_lowering=False, debug=False)
    xT = nc.dram_tensor("xT", [C, T], bf16, kind="ExternalInput").ap()
    wqk = nc.dram_tensor("wqk", [C, 2 * F], bf16, kind="ExternalInput").ap()
    wv = nc.dram_tensor("wv", [C, F], bf16, kind="ExternalInput").ap()
    wo = nc.dram_tensor("wo", [F, C], bf16, kind="ExternalInput").ap()
    yT = nc.dram_tensor("yT", [C, T], f32, kind="ExternalOutput").ap()

    with tile.TileContext(nc) as tc:
        with (
            tc.tile_pool(name="consts", bufs=1) as consts,
            tc.tile_pool(name="kv", bufs=1) as kv,
            tc.tile_pool(name="qtp", bufs=2) as qtp,
            tc.tile_pool(name="win", bufs=1) as win,
            tc.tile_pool(name="xin", bufs=1) as xin,
            tc.tile_pool(name="wout", bufs=1) as wout,
            tc.tile_pool(name="obuf", bufs=8) as obuf,
            tc.tile_pool(name="oun", bufs=2) as oun,
            tc.tile_pool(name="pexp", bufs=4) as pexp,
            tc.tile_pool(name="rcp", bufs=2) as rcp,
            tc.tile_pool(name="tmpb", bufs=2) as tmpb,
            tc.tile_pool(name="ysb", bufs=2) as ysb,
            tc.tile_pool(name="ps", bufs=2, space="PSUM") as ps,
            tc.tile_pool(name="pss", bufs=2, space="PSUM") as pss,
            tc.tile_pool(name="pso", bufs=2, space="PSUM") as pso,
        ):
            tri = consts.tile([128, 128], f32)
            make_upper_triangular(nc, tri[:], val=1.0, diag=True)

            kt_sb = kv.tile([128, 4, T], bf16)  # K^T; head 2p|2p+1 on parts 0-63|64-127
            # V aug ([t, head, d] + ones col): stationary operand of the
            # AV matmul; bf16 error passes linearly into O (~0.2% rel).
            v_sb = kv.tile([128, NKT, HPC, HD + 1], bf16)
            ones = consts.tile([128, NKT * HPC], f32)
            nc.vector.memset(ones[:], 1.0)
            nc.vector.tensor_copy(
                out=v_sb[:, :, :, HD : HD + 1],
                in_=ones[:].rearrange("p (a b c) -> p a b c", a=NKT, b=HPC),
            )
            # ones row for the denominator broadcast matmul (K=1, M=64).
            # [65, 64] so row 64 can be sliced: the matmul's lhsT must sit on
            # the same base partition as the denominator row of o_un (= HD).
            ones_row = consts.tile([HD + 1, 64], f32r)
            nc.vector.tensor_copy(out=ones_row[:], in_=ones[0 : HD + 1, 0:64])

            # ~4.5us of dummy matmuls at t=0: spins the PE through the HAM
            # activity window while the first input DMAs land, so phase1
            # starts at 2.4GHz instead of ramping from the 1.2GHz cold state.
            warm = consts.tile([128, CH], bf16)
            nc.vector.memset(warm[:], 1.0)
            wps = ps.tile([128, CH], f32, name="warmps", tag="mm")
            for i in range(10):
                nc.tensor.matmul(
                    wps[:], warm[:, 0:128], warm[:], start=True, stop=True
                )

            # weights: split + interleave DMAs per k-tile so the first
            # matmul's operands (wqk[0], xt[0]) land within ~1us
            xt_first = xin.tile([128, 8, CH], bf16, name="xt0", tag="xt")
            wqk_sb = win.tile([128, 8, 2 * F], bf16)
            wv_sb = win.tile([128, 8, F], bf16)
            for kt in range(8):
                nc.sync.dma_start(
                    out=wqk_sb[:, kt, :], in_=wqk[kt * 128 : (kt + 1) * 128, :]
                )
                nc.sync.dma_start(
                    out=xt_first[:, kt, :],
                    in_=xT[kt * 128 : (kt + 1) * 128, 0:CH],
                )
            for kt in range(8):
                nc.sync.dma_start(
                    out=wv_sb[:, kt, :], in_=wv[kt * 128 : (kt + 1) * 128, :]
                )
            wo_sb = wout.tile([128, 4, C], bf16)

            qt_tiles = {}
            osb_tiles = {}

            def phase1(j):
                t0 = j * CH
                if j == 0:
                    xt = xt_first
                else:
                    xt = xin.tile([128, 8, CH], bf16, name=f"xt{j}", tag="xt")
                    nc.sync.dma_start(
                        out=xt[:],
                        in_=xT[:, t0 : t0 + CH].rearrange("(kt p) t -> p kt t", p=128),
                    )
                qt = qtp.tile([128, 4, CH], bf16, name=f"qt{j}", tag="qt")
                qt_tiles[j] = qt
                for m in range(8):
                    acc = ps.tile([128, CH], f32, name=f"qk_{j}_{m}", tag="mm")
                    for kt in range(8):
                        nc.tensor.matmul(
                            acc[:],
                            wqk_sb[:, kt, m * 128 : (m + 1) * 128],
                            xt[:, kt, :],
                            start=(kt == 0),
                            stop=(kt == 7),
                        )
                    if m < 4:
                        nc.vector.tensor_copy(out=qt[:, m, :], in_=acc[:])
                    else:
                        nc.vector.tensor_copy(
                            out=kt_sb[:, m % 4, t0 : t0 + CH], in_=acc[:]
                        )
                for s in range(CH // 128):
                    accv = ps.tile([128, F], f32, name=f"v_{j}_{s}", tag="mm")
                    for kt in range(8):
                        nc.tensor.matmul(
                            accv[:],
                            xt[:, kt, s * 128 : (s + 1) * 128],
                            wv_sb[:, kt, :],
                            start=(kt == 0),
                            stop=(kt == 7),
                        )
                    nc.vector.tensor_copy(
                        out=v_sb[:, 4 * j + s, :, 0:HD],
                        in_=accv[:].rearrange("p (h d) -> p h d", h=HPC),
                    )

            def phase2(j):
                q0 = j * CH
                nkt = 4 * j + 4
                osb_tiles[j] = []
                pending_norm = [None]

                def emit_norm():
                    if pending_norm[0] is None:
                        return
                    o_un, o_sb, tag = pending_norm[0]
                    pending_norm[0] = None
                    # broadcast the denominator row down 64 partitions with a
                    # K=1 matmul, reciprocal it lane-parallel out of PSUM,
                    # multiply into O^T. All on-chip; no DRAM bounce.
                    # den borrows the ps pool: its previous occupants' readers
                    # are always emitted in an earlier phase block, so the
                    # WAR wait can't deadlock the PE queue (pso would).
                    rinv = rcp.tile([64, 2, CH], f32, name=f"ri{tag}", tag="ri")
                    for hf in range(2):
                        den = ps.tile([64, CH], f32, name=f"dn{tag}_{hf}", tag="mm")
                        nc.tensor.matmul(
                            den[:],
                            ones_row[HD : HD + 1, :],
                            o_un[HD : HD + 1, hf, :],
                            start=True,
                            stop=True,
                        )
                        nc.vector.reciprocal(out=rinv[:, hf, :], in_=den[:])
                    nc.vector.tensor_mul(
                        out=o_sb[0:64, :],
                        in0=o_un[0:HD, 0, :],
                        in1=rinv[:, 0, :],
                    )
                    tmp = tmpb.tile([64, CH], bf16, name=f"tm{tag}", tag="tm")
                    nc.vector.tensor_mul(
                        out=tmp[:], in0=o_un[0:HD, 1, :], in1=rinv[:, 1, :]
                    )
                    nc.sync.dma_start(out=o_sb[64:128, :], in_=tmp[:])

                for hp in range(4):
                    o_sb = obuf.tile(
                        [128, CH], bf16, name=f"osb{j}_{hp}", tag="osb"
                    )
                    osb_tiles[j].append(o_sb)
                    o_un = oun.tile(
                        [HD + 1, 2, CH], f32r, name=f"oun_{j}_{hp}", tag="oun"
                    )
                    o_ps = [
                        pso.tile([HD + 1, CH], f32, name=f"o_{j}_{hp}_{hf}", tag="o")
                        for hf in range(2)
                    ]

                    def av(kt, p_t, lo):
                        for hf in range(2):
                            h = 2 * hp + hf
                            nc.tensor.matmul(
                                o_ps[hf][:, lo:CH],
                                v_sb[:, kt, h, :],
                                p_t[:, hf, lo:CH],
                                start=(kt == 0),
                                stop=(kt == nkt - 1),
                            )

                    prev = None
                    for kt in range(nkt):
                        k0 = kt * 128
                        lo = max(k0 - q0, 0)
                        # both heads' scores in one 2-bank PSUM tile so a
                        # single wide exp covers the pair; the two K=64
                        # matmuls sit on disjoint PE row groups (0-63 /
                        # 64-127) and are emitted back-to-back so they run
                        # concurrently in the array.
                        s_t = pss.tile(
                            [128, 2, CH], f32, name=f"s_{j}_{hp}_{kt}", tag="s"
                        )
                        p_t = pexp.tile(
                            [128, 2, CH],
                            bf16,
                            name=f"p_{j}_{hp}_{kt}",
                            tag="p",
                        )
                        for hf in range(2):
                            pb = hf * 64
                            nc.tensor.matmul(
                                s_t[:, hf, lo:CH],
                                kt_sb[pb : pb + 64, hp, k0 : k0 + 128],
                                qt_tiles[j][pb : pb + 64, hp, lo:CH],
                                start=True,
                                stop=True,
                            )
                        nc.scalar.activation(
                            out=p_t[:, :, lo:CH],
                            in_=s_t[:, :, lo:CH],
                            func=mybir.ActivationFunctionType.Exp,
                            scale=0.125,
                        )
                        if k0 >= q0:
                            for hf in range(2):
                                nc.vector.tensor_mul(
                                    out=p_t[:, hf, lo : lo + 128],
                                    in0=p_t[:, hf, lo : lo + 128],
                                    in1=tri[:],
                                )
                        # AV runs one k-tile behind so exp(kt) overlaps it
                        if prev is not None:
                            av(*prev)
                        if kt == 1:
                            # previous head-pair's normalization: emitted here
                            # so its den matmul sits behind already-ready QK
                            # work in the PE queue instead of stalling it.
                            emit_norm()
                        prev = (kt, p_t, lo)
                    av(*prev)
                    for hf in range(2):
                        nc.vector.tensor_copy(
                            out=o_un[:, hf, :], in_=o_ps[hf][:, :]
                        )
                    pending_norm[0] = (o_un, o_sb, f"{j}_{hp}")
                # the last head-pair's normalization is returned to the
                # caller, which flushes it after emitting the next phase's
                # (already data-ready) matmuls -- keeps the den matmul from
                # parking at the PE queue head waiting on the o_un copy.
                return emit_norm

            def phase3(j):
                q0 = j * CH
                for ot in range(8):
                    acc = ps.tile([128, CH], f32, name=f"y_{j}_{ot}", tag="mm")
                    for ft in range(4):
                        nc.tensor.matmul(
                            acc[:],
                            wo_sb[:, ft, ot * 128 : (ot + 1) * 128],
                            osb_tiles[j][ft][:, :],
                            start=(ft == 0),
                            stop=(ft == 3),
                        )
                    y = ysb.tile([128, CH], f32, name=f"ysb_{j}_{ot}", tag="y")
                    nc.vector.tensor_copy(out=y[:], in_=acc[:])
                    nc.sync.dma_start(
                        out=yT[ot * 128 : (ot + 1) * 128, q0 : q0 + CH], in_=y[:]
                    )

            # interleaved emission; phase3 runs one chunk behind phase2 so the
            # normalization pipeline hides under the next chunk's compute
            phase1(0)
            phase1(1)
            n0 = phase2(0)
            # wo is first needed by phase3(0); emitting its DMA here keeps
            # 1MB out of the startup queue ahead of xt(1)
            nc.sync.dma_start(
                out=wo_sb[:], in_=wo.rearrange("(ft p) o -> p ft o", p=128)
            )
            phase1(2)
            n0()
            n1 = phase2(1)
            phase3(0)
            phase1(3)
            n1()
            n2 = phase2(2)
            phase3(1)
            n2()
            n3 = phase2(3)
            phase3(2)
            n3()
            phase3(3)

    nc.compile()
    return nc


def to_bf16(a: np.ndarray) -> np.ndarray:
    return np.ascontiguousarray(a, dtype=np.float32).astype(ml_dtypes.bfloat16)


def shard_inputs(x, W_qkv, W_out):
    """Build the 8 per-core input maps."""
    xT = [to_bf16(x[b].T) for b in range(B)]
    maps = []
    for core in range(NCORES):
        b, hf = core // 2, core % 2
        wq = W_qkv[:, hf * F : (hf + 1) * F]
        wk = W_qkv[:, C + hf * F : C + (hf + 1) * F]
        wv = W_qkv[:, 2 * C + hf * F : 2 * C + (hf + 1) * F]
        maps.append(
            {
                "xT": xT[b],
                "wqk": to_bf16(np.concatenate([wq, wk], axis=1)),
                "wv": to_bf16(wv),
                "wo": to_bf16(W_out[hf * F : (hf + 1) * F, :]),
            }
        )
    return maps


_NC_CACHE = {}


def get_nc():
    if "nc" not in _NC_CACHE:
        _NC_CACHE["nc"] = build_nc()
    return _NC_CACHE["nc"]


def kernel(x, W_qkv, W_out, _run_kwargs=None):
    x = np.asarray(x, dtype=np.float32)
    W_qkv = np.asarray(W_qkv, dtype=np.float32)
    W_out = np.asarray(W_out, dtype=np.float32)
    nc = get_nc()
    maps = shard_inputs(x, W_qkv, W_out)
    res = run_bass_kernel_spmd(nc, maps, list(range(NCORES)), **(_run_kwargs or {}))
    out = np.empty((B, T, C), dtype=np.float32)
    for b in range(B):
        yT0 = res.results[2 * b]["yT"]
        yT1 = res.results[2 * b + 1]["yT"]
        out[b] = (yT0 + yT1).T
    if _run_kwargs is not None:
        _NC_CACHE["last_results"] = res
    return out


# revision 11
# speedup vs baseline: 1.1350x; 1.1350x over previous
"""Causal self-attention (B=4, T=2048, C=1024, H=16) on 8 TRN2 NeuronCores.

Sharding: batch x head-halves. Core i handles batch b=i//2 and heads
[8*(i%2), 8*(i%2)+8). Each core computes QKV projection for its slice,
causal attention for its 8 heads, and a partial output projection
(512 of 1024 contraction features). The host sums the two partials per
batch and transposes back.

All matmul operands are bf16 (fp32 PSUM accumulation): 1 cycle/row on
the PE with no small-N penalty, and half the DMA/SBUF footprint of
fp32 -- the startup weight+activation load that gates the first real
matmuls drops from 6MB to 3MB, keeping the PE out of its cold
(1.2GHz) HAM state sooner.

Attention per head works on S^T tiles: S^T[k, q] so that the AV matmul
(lhsT = V [k, d+1], rhs = P^T [k, q]) needs no transposes, with a ones
column appended to V so row 64 of the PSUM accumulator collects the
softmax denominators. The two heads of a pair run their QK^T matmuls
concurrently in the PE array (disjoint 64-row groups). exp happens on
the scalar engine PSUM->SBUF with the 1/8 scale folded in; no max
subtraction (scores are ~N(0,1), safe in fp32). Causality: (k-tile,
q-chunk) pairs above the diagonal are skipped, diagonal tiles compute
only the valid columns and get a triangular mask multiply.

Softmax normalization stays on-chip: the denominator row of o_un is
broadcast down 64 partitions with a K=1 matmul against a ones row
(lhsT [1,64]), reciprocal'd lane-parallel on the vector engine
straight out of PSUM, and multiplied into O^T. No DRAM bounce, no
partition-scatter DMAs. Each head-pair's normalization is emitted one
head-pair late so the tiny broadcast matmul never parks at the head of
the PE queue waiting for the o_un copy.

Pipelining: phase1 (projections, per 512-t-chunk), phase2 (attention,
per 512-q-chunk) and phase3 (out-proj, one chunk behind) are emitted
interleaved -- legal because causal attention for q-chunk j only needs
K/V of chunks <= j.
"""

import sys

if "/opt/trn_rl_repo" not in sys.path:
    sys.path.insert(0, "/opt/trn_rl_repo")

import numpy as np
import ml_dtypes

import concourse.bass as bass
import concourse.mybir as mybir
import concourse.tile as tile
from concourse import bacc
from concourse.bass_utils import run_bass_kernel_spmd
from concourse.masks import make_upper_triangular

B, T, C, H = 4, 2048, 1024, 16
HD = C // H  # 64
NCORES = 8
HPC = H // 2  # heads per core = 8
F = HPC * HD  # 512 features per core
CH = 512  # t/q chunk width
NCH = T // CH  # 4
NKT = T // 128  # 16 k-tiles

f32 = mybir.dt.float32
f32r = mybir.dt.float32r
bf16 = mybir.dt.bfloat16


def build_nc():
    nc = bacc.Bacc("TRN2", target_bir_lowering=False, debug=False)
    xT = nc.dram_tensor("xT", [C, T], bf16, kind="ExternalInput").ap()
    wqk = nc.dram_tensor("wqk", [C, 2 * F], bf16, kind="ExternalInput").ap()
    wv = nc.dram_tensor("wv", [C, F], bf16, kind="ExternalInput").ap()
    wo = nc.dram_tensor("wo", [F, C], bf16, kind="ExternalInput").ap()
    yT = nc.dram_tensor("yT", [C, T], f32, kind="ExternalOutput").ap()

    with tile.TileContext(nc) as tc:
        with (
            tc.tile_pool(name="consts", bufs=1) as consts,
            tc.tile_pool(name="kv", bufs=1) as kv,
            tc.tile_pool(name="qtp", bufs=2) as qtp,
            tc.tile_pool(name="win", bufs=1) as win,
            tc.tile_pool(name="xin", bufs=1) as xin,
            tc.tile_pool(name="wout", bufs=1) as wout,
            tc.tile_pool(name="obuf", bufs=8) as obuf,
            tc.tile_pool(name="oun", bufs=2) as oun,
            tc.tile_pool(name="pexp", bufs=4) as pexp,
            tc.tile_pool(name="rcp", bufs=2) as rcp,
            tc.tile_pool(name="tmpb", bufs=2) as tmpb,
            tc.tile_pool(name="ysb", bufs=2) as ysb,
            tc.tile_pool(name="ps", bufs=2, space="PSUM") as ps,
            tc.tile_pool(name="pss", bufs=2, space="PSUM") as pss,
            tc.tile_pool(name="pso", bufs=2, space="PSUM") as pso,
        ):
            tri = consts.tile([128, 128], f32)
            make_upper_triangular(nc, tri[:], val=1.0, diag=True)

            kt_sb = kv.tile([128, 4, T], bf16)  # K^T; head 2p|2p+1 on parts 0-63|64-127
            # V aug ([t, head, d] + ones col): stationary operand of the
            # AV matmul; bf16 error passes linearly into O (~0.2% rel).
            v_sb = kv.tile([128, NKT, HPC, HD + 1], bf16)
            ones = consts.tile([128, NKT * HPC], f32)
            nc.vector.memset(ones[:], 1.0)
            nc.vector.tensor_copy(
                out=v_sb[:, :, :, HD : HD + 1],
                in_=ones[:].rearrange("p (a b c) -> p a b c", a=NKT, b=HPC),
            )
            # ones row for the denominator broadcast matmul (K=1, M=64).
            # [65, 64] so row 64 can be sliced: the matmul's lhsT must sit on
            # the same base partition as the denominator row of o_un (= HD).
            ones_row = consts.tile([HD + 1, 64], f32r)
            nc.vector.tensor_copy(out=ones_row[:], in_=ones[0 : HD + 1, 0:64])

            # ~4.5us of dummy matmuls at t=0: spins the PE through the HAM
            # activity window while the first input DMAs land, so phase1
            # starts at 2.4GHz instead of ramping from the 1.2GHz cold state.
            warm = consts.tile([128, CH], bf16)
            nc.vector.memset(warm[:], 1.0)
            wps = ps.tile([128, CH], f32, name="warmps", tag="mm")
            for i in range(10):
                nc.tensor.matmul(
                    wps[:], warm[:, 0:128], warm[:], start=True, stop=True
                )

            # weights: split + interleave DMAs per k-tile so the first
            # matmul's operands (wqk[0], xt[0]) land within ~1us
            xt_first = xin.tile([128, 8, CH], bf16, name="xt0", tag="xt")
            wqk_sb = win.tile([128, 8, 2 * F], bf16)
            wv_sb = win.tile([128, 8, F], bf16)
            for kt in range(8):
                nc.sync.dma_start(
                    out=wqk_sb[:, kt, :], in_=wqk[kt * 128 : (kt + 1) * 128, :]
                )
                nc.sync.dma_start(
                    out=xt_first[:, kt, :],
                    in_=xT[kt * 128 : (kt + 1) * 128, 0:CH],
                )
            for kt in range(8):
                nc.sync.dma_start(
                    out=wv_sb[:, kt, :], in_=wv[kt * 128 : (kt + 1) * 128, :]
                )
            wo_sb = wout.tile([128, 4, C], bf16)

            qt_tiles = {}
            osb_tiles = {}

            def phase1(j):
                t0 = j * CH
                if j == 0:
                    xt = xt_first
                else:
                    xt = xin.tile([128, 8, CH], bf16, name=f"xt{j}", tag="xt")
                    nc.sync.dma_start(
                        out=xt[:],
                        in_=xT[:, t0 : t0 + CH].rearrange("(kt p) t -> p kt t", p=128),
                    )
                qt = qtp.tile([128, 4, CH], bf16, name=f"qt{j}", tag="qt")
                qt_tiles[j] = qt
                for m in range(8):
                    acc = ps.tile([128, CH], f32, name=f"qk_{j}_{m}", tag="mm")
                    for kt in range(8):
                        nc.tensor.matmul(
                            acc[:],
                            wqk_sb[:, kt, m * 128 : (m + 1) * 128],
                            xt[:, kt, :],
                            start=(kt == 0),
                            stop=(kt == 7),
                        )
                    if m < 4:
                        nc.vector.tensor_copy(out=qt[:, m, :], in_=acc[:])
                    else:
                        nc.vector.tensor_copy(
                            out=kt_sb[:, m % 4, t0 : t0 + CH], in_=acc[:]
                        )
                for s in range(CH // 128):
                    accv = ps.tile([128, F], f32, name=f"v_{j}_{s}", tag="mm")
                    for kt in range(8):
                        nc.tensor.matmul(
                            accv[:],
                            xt[:, kt, s * 128 : (s + 1) * 128],
                            wv_sb[:, kt, :],
                            start=(kt == 0),
                            stop=(kt == 7),
                        )
                    nc.vector.tensor_copy(
                        out=v_sb[:, 4 * j + s, :, 0:HD],
                        in_=accv[:].rearrange("p (h d) -> p h d", h=HPC),
                    )

            def phase2(j):
                q0 = j * CH
                nkt = 4 * j + 4
                osb_tiles[j] = []
                pending_norm = [None]

                def emit_norm():
                    if pending_norm[0] is None:
                        return
                    o_un, o_sb, tag = pending_norm[0]
                    pending_norm[0] = None
                    # broadcast the denominator row down 64 partitions with a
                    # K=1 matmul, reciprocal it lane-parallel out of PSUM,
                    # multiply into O^T. All on-chip; no DRAM bounce.
                    # den borrows the ps pool: its previous occupants' readers
                    # are always emitted in an earlier phase block, so the
                    # WAR wait can't deadlock the PE queue (pso would).
                    rinv = rcp.tile([64, 2, CH], f32, name=f"ri{tag}", tag="ri")
                    for hf in range(2):
                        den = ps.tile([64, CH], f32, name=f"dn{tag}_{hf}", tag="mm")
                        nc.tensor.matmul(
                            den[:],
                            ones_row[HD : HD + 1, :],
                            o_un[HD : HD + 1, hf, :],
                            start=True,
                            stop=True,
                        )
                        nc.vector.reciprocal_approx_fast(
                            out=rinv[:, hf, :], in_=den[:]
                        )
                    nc.vector.tensor_mul(
                        out=o_sb[0:64, :],
                        in0=o_un[0:HD, 0, :],
                        in1=rinv[:, 0, :],
                    )
                    tmp = tmpb.tile([64, CH], bf16, name=f"tm{tag}", tag="tm")
                    nc.vector.tensor_mul(
                        out=tmp[:], in0=o_un[0:HD, 1, :], in1=rinv[:, 1, :]
                    )
                    nc.sync.dma_start(out=o_sb[64:128, :], in_=tmp[:])

                for hp in range(4):
                    o_sb = obuf.tile(
                        [128, CH], bf16, name=f"osb{j}_{hp}", tag="osb"
                    )
                    osb_tiles[j].append(o_sb)
                    o_un = oun.tile(
                        [HD + 1, 2, CH], f32r, name=f"oun_{j}_{hp}", tag="oun"
                    )
                    o_ps = [
                        pso.tile([HD + 1, CH], f32, name=f"o_{j}_{hp}_{hf}", tag="o")
                        for hf in range(2)
                    ]

                    def av(kt, p_t, lo):
                        for hf in range(2):
                            h = 2 * hp + hf
                            nc.tensor.matmul(
                                o_ps[hf][:, lo:CH],
                                v_sb[:, kt, h, :],
                                p_t[:, hf, lo:CH],
                                start=(kt == 0),
                                stop=(kt == nkt - 1),
                            )

                    prev = None
                    for kt in range(nkt):
                        k0 = kt * 128
                        lo = max(k0 - q0, 0)
                        # both heads' scores in one 2-bank PSUM tile so a
                        # single wide exp covers the pair; the two K=64
                        # matmuls sit on disjoint PE row groups (0-63 /
                        # 64-127) and are emitted back-to-back so they run
                        # concurrently in the array.
                        s_t = pss.tile(
                            [128, 2, CH], f32, name=f"s_{j}_{hp}_{kt}", tag="s"
                        )
                        p_t = pexp.tile(
                            [128, 2, CH],
                            bf16,
                            name=f"p_{j}_{hp}_{kt}",
                            tag="p",
                        )
                        for hf in range(2):
                            pb = hf * 64
                            nc.tensor.matmul(
                                s_t[:, hf, lo:CH],
                                kt_sb[pb : pb + 64, hp, k0 : k0 + 128],
                                qt_tiles[j][pb : pb + 64, hp, lo:CH],
                                start=True,
                                stop=True,
                            )
                        nc.scalar.activation(
                            out=p_t[:, :, lo:CH],
                            in_=s_t[:, :, lo:CH],
                            func=mybir.ActivationFunctionType.Exp,
                            scale=0.125,
                        )
                        if k0 >= q0:
                            for hf in range(2):
                                nc.vector.tensor_mul(
                                    out=p_t[:, hf, lo : lo + 128],
                                    in0=p_t[:, hf, lo : lo + 128],
                                    in1=tri[:],
                                )
                        # AV runs one k-tile behind so exp(kt) overlaps it
                        if prev is not None:
                            av(*prev)
                        if kt == 1:
                            # previous head-pair's normalization: emitted here
                            # so its den matmul sits behind already-ready QK
                            # work in the PE queue instead of stalling it.
                            emit_norm()
                        prev = (kt, p_t, lo)
                    av(*prev)
                    for hf in range(2):
                        nc.vector.tensor_copy(
                            out=o_un[:, hf, :], in_=o_ps[hf][:, :]
                        )
                    pending_norm[0] = (o_un, o_sb, f"{j}_{hp}")
                # the last head-pair's normalization is returned to the
                # caller, which flushes it after emitting the next phase's
                # (already data-ready) matmuls -- keeps the den matmul from
                # parking at the PE queue head waiting on the o_un copy.
                return emit_norm

            def phase3(j):
                q0 = j * CH
                for ot in range(8):
                    acc = ps.tile([128, CH], f32, name=f"y_{j}_{ot}", tag="mm")
                    for ft in range(4):
                        nc.tensor.matmul(
                            acc[:],
                            wo_sb[:, ft, ot * 128 : (ot + 1) * 128],
                            osb_tiles[j][ft][:, :],
                            start=(ft == 0),
                            stop=(ft == 3),
                        )
                    y = ysb.tile([128, CH], f32, name=f"ysb_{j}_{ot}", tag="y")
                    nc.vector.tensor_copy(out=y[:], in_=acc[:])
                    nc.sync.dma_start(
                        out=yT[ot * 128 : (ot + 1) * 128, q0 : q0 + CH], in_=y[:]
                    )

            # interleaved emission; phase3 runs one chunk behind phase2 so the
            # normalization pipeline hides under the next chunk's compute
            phase1(0)
            phase1(1)
            n0 = phase2(0)
            # wo is first needed by phase3(0); emitting its DMA here keeps
            # 1MB out of the startup queue ahead of xt(1)
            nc.sync.dma_start(
                out=wo_sb[:], in_=wo.rearrange("(ft p) o -> p ft o", p=128)
            )
            phase1(2)
            n0()
            n1 = phase2(1)
            phase3(0)
            phase1(3)
            n1()
            n2 = phase2(2)
            phase3(1)
            n2()
            n3 = phase2(3)
            phase3(2)
            n3()
            phase3(3)

    nc.compile()
    return nc


def to_bf16(a: np.ndarray) -> np.ndarray:
    return np.ascontiguousarray(a, dtype=np.float32).astype(ml_dtypes.bfloat16)


def shard_inputs(x, W_qkv, W_out):
    """Build the 8 per-core input maps."""
    xT = [to_bf16(x[b].T) for b in range(B)]
    maps = []
    for core in range(NCORES):
        b, hf = core // 2, core % 2
        wq = W_qkv[:, hf * F : (hf + 1) * F]
        wk = W_qkv[:, C + hf * F : C + (hf + 1) * F]
        wv = W_qkv[:, 2 * C + hf * F : 2 * C + (hf + 1) * F]
        maps.append(
            {
                "xT": xT[b],
                "wqk": to_bf16(np.concatenate([wq, wk], axis=1)),
                "wv": to_bf16(wv),
                "wo": to_bf16(W_out[hf * F : (hf + 1) * F, :]),
            }
        )
    return maps


_NC_CACHE = {}


def get_nc():
    if "nc" not in _NC_CACHE:
        _NC_CACHE["nc"] = build_nc()
    return _NC_CACHE["nc"]


def kernel(x, W_qkv, W_out, _run_kwargs=None):
    x = np.asarray(x, dtype=np.float32)
    W_qkv = np.asarray(W_qkv, dtype=np.float32)
    W_out = np.asarray(W_out, dtype=np.float32)
    nc = get_nc()
    maps = shard_inputs(x, W_qkv, W_out)
    res = run_bass_kernel_spmd(nc, maps, list(range(NCORES)), **(_run_kwargs or {}))
    out = np.empty((B, T, C), dtype=np.float32)
    for b in range(B):
        yT0 = res.results[2 * b]["yT"]
        yT1 = res.results[2 * b + 1]["yT"]
        out[b] = (yT0 + yT1).T
    if _run_kwargs is not None:
        _NC_CACHE["last_results"] = res
    return out


# revision 14
# speedup vs baseline: 1.2227x; 1.0773x over previous
"""Causal self-attention (B=4, T=2048, C=1024, H=16) on 8 TRN2 NeuronCores.

Sharding: batch x head-halves. Core i handles batch b=i//2 and heads
[8*(i%2), 8*(i%2)+8). Each core computes QKV projection for its slice,
causal attention for its 8 heads, and a partial output projection
(512 of 1024 contraction features). The host sums the two partials per
batch and transposes back.

All matmul operands are bf16 (fp32 PSUM accumulation): 1 cycle/row on
the PE with no small-N penalty, and half the DMA/SBUF footprint of
fp32 -- the startup weight+activation load that gates the first real
matmuls drops from 6MB to 3MB, keeping the PE out of its cold
(1.2GHz) HAM state sooner.

Attention per head works on S^T tiles: S^T[k, q] so that the AV matmul
(lhsT = V [k, d+1], rhs = P^T [k, q]) needs no transposes, with a ones
column appended to V so row 64 of the PSUM accumulator collects the
softmax denominators. The two heads of a pair run their QK^T matmuls
concurrently in the PE array (disjoint 64-row groups). exp happens on
the scalar engine PSUM->SBUF with the 1/8 scale folded in; no max
subtraction (scores are ~N(0,1), safe in fp32). Causality: (k-tile,
q-chunk) pairs above the diagonal are skipped, diagonal tiles compute
only the valid columns and get a triangular mask multiply.

Softmax normalization stays on-chip: the denominator row of o_un is
broadcast down 64 partitions with a K=1 matmul against a ones row,
reciprocal'd lane-parallel (approx-fast, ~18 bits) straight out of
PSUM, and multiplied into O^T. No DRAM bounce. Each head-pair's
normalization is emitted one head-pair late so the tiny broadcast
matmul never parks at the head of the PE queue waiting on the o_un
copy.

Scheduling: the attention inner loop is exp-paced (the scalar engine
needs ~985ns per k-tile vs ~640ns of PE work), so projection work is
*woven into* phase2's emission stream as quanta -- one QKV-projection
m-block or one out-projection column tile at a time -- pulled mid
head-pair and after each head-pair. This keeps the PE queue stocked
with ready matmuls through the exp-paced stretches, which both fills
the idle cycles and stops the HAM clock gate from re-throttling the
array to 1.2GHz mid-kernel.
"""

import sys

if "/opt/trn_rl_repo" not in sys.path:
    sys.path.insert(0, "/opt/trn_rl_repo")

import numpy as np
import ml_dtypes

import concourse.bass as bass
import concourse.mybir as mybir
import concourse.tile as tile
from concourse import bacc
from concourse.bass_utils import run_bass_kernel_spmd
from concourse.masks import make_upper_triangular

B, T, C, H = 4, 2048, 1024, 16
HD = C // H  # 64
NCORES = 8
HPC = H // 2  # heads per core = 8
F = HPC * HD  # 512 features per core
CH = 512  # t/q chunk width
NCH = T // CH  # 4
NKT = T // 128  # 16 k-tiles

f32 = mybir.dt.float32
f32r = mybir.dt.float32r
bf16 = mybir.dt.bfloat16


def build_nc():
    nc = bacc.Bacc("TRN2", target_bir_lowering=False, debug=False)
    xT = nc.dram_tensor("xT", [C, T], bf16, kind="ExternalInput").ap()
    wqk = nc.dram_tensor("wqk", [C, 2 * F], bf16, kind="ExternalInput").ap()
    wv = nc.dram_tensor("wv", [C, F], bf16, kind="ExternalInput").ap()
    wo = nc.dram_tensor("wo", [F, C], bf16, kind="ExternalInput").ap()
    yT = nc.dram_tensor("yT", [C, T], f32, kind="ExternalOutput").ap()

    with tile.TileContext(nc) as tc:
        with (
            tc.tile_pool(name="consts", bufs=1) as consts,
            tc.tile_pool(name="kv", bufs=1) as kv,
            # all 4 qt tiles coexist: qt(j+2)'s projection copies are woven
            # into phase2(j)'s emission, so reusing qt(j)'s buffer would make
            # that copy wait on phase2(j) QK matmuls emitted *after* it -- a
            # cross-queue deadlock. 4 buffers removes the rotation entirely.
            tc.tile_pool(name="qtp", bufs=4) as qtp,
            tc.tile_pool(name="win", bufs=1) as win,
            tc.tile_pool(name="xin", bufs=2) as xin,
            tc.tile_pool(name="wout", bufs=1) as wout,
            tc.tile_pool(name="obuf", bufs=8) as obuf,
            tc.tile_pool(name="oun", bufs=2) as oun,
            tc.tile_pool(name="pexp", bufs=4) as pexp,
            tc.tile_pool(name="rcp", bufs=2) as rcp,
            tc.tile_pool(name="tmpb", bufs=2) as tmpb,
            tc.tile_pool(name="ysb", bufs=2) as ysb,
            tc.tile_pool(name="ps", bufs=2, space="PSUM") as ps,
            tc.tile_pool(name="pss", bufs=2, space="PSUM") as pss,
            tc.tile_pool(name="pso", bufs=2, space="PSUM") as pso,
        ):
            tri = consts.tile([128, 128], f32)
            make_upper_triangular(nc, tri[:], val=1.0, diag=True)

            kt_sb = kv.tile([128, 4, T], bf16)  # K^T; head 2p|2p+1 on parts 0-63|64-127
            # V aug ([t, head, d] + ones col): stationary operand of the
            # AV matmul; bf16 error passes linearly into O (~0.2% rel).
            v_sb = kv.tile([128, NKT, HPC, HD + 1], bf16)
            ones = consts.tile([128, NKT * HPC], f32)
            nc.vector.memset(ones[:], 1.0)
            nc.vector.tensor_copy(
                out=v_sb[:, :, :, HD : HD + 1],
                in_=ones[:].rearrange("p (a b c) -> p a b c", a=NKT, b=HPC),
            )
            # ones row for the denominator broadcast matmul (K=1, M=64).
            # [65, 64] so row 64 can be sliced: the matmul's lhsT must sit on
            # the same base partition as the denominator row of o_un (= HD).
            ones_row = consts.tile([HD + 1, 64], f32r)
            nc.vector.tensor_copy(out=ones_row[:], in_=ones[0 : HD + 1, 0:64])

            # ~7us of dummy matmuls at t=0: spins the PE through the HAM
            # activity window while the first input DMAs land, so phase1
            # starts at 2.4GHz instead of ramping from the 1.2GHz cold state.
            warm = consts.tile([128, CH], bf16)
            nc.vector.memset(warm[:], 1.0)
            wps = ps.tile([128, CH], f32, name="warmps", tag="mm")
            for i in range(16):
                nc.tensor.matmul(
                    wps[:], warm[:, 0:128], warm[:], start=True, stop=True
                )

            # weights: split + interleave DMAs per k-tile so the first
            # matmul's operands (wqk[0], xt[0]) land within ~1us
            xt_tiles = {}
            xt_tiles[0] = xin.tile([128, 8, CH], bf16, name="xt0", tag="xt")
            wqk_sb = win.tile([128, 8, 2 * F], bf16)
            wv_sb = win.tile([128, 8, F], bf16)
            for kt in range(8):
                nc.sync.dma_start(
                    out=wqk_sb[:, kt, :], in_=wqk[kt * 128 : (kt + 1) * 128, :]
                )
                nc.sync.dma_start(
                    out=xt_tiles[0][:, kt, :],
                    in_=xT[kt * 128 : (kt + 1) * 128, 0:CH],
                )
            for kt in range(8):
                nc.sync.dma_start(
                    out=wv_sb[:, kt, :], in_=wv[kt * 128 : (kt + 1) * 128, :]
                )
            wo_sb = wout.tile([128, 4, C], bf16)

            def prefetch_xt(j):
                # emitted well before phase1(j)'s quanta so the DMA lands
                # before any matmul in the FIFO waits on it
                xt = xin.tile([128, 8, CH], bf16, name=f"xt{j}", tag="xt")
                xt_tiles[j] = xt
                t0 = j * CH
                nc.sync.dma_start(
                    out=xt[:],
                    in_=xT[:, t0 : t0 + CH].rearrange("(kt p) t -> p kt t", p=128),
                )

            qt_tiles = {}
            osb_tiles = {}
            pending_norm = [None]

            def emit_norm():
                if pending_norm[0] is None:
                    return
                o_un, o_sb, tag = pending_norm[0]
                pending_norm[0] = None
                # broadcast the denominator row down 64 partitions with a
                # K=1 matmul, reciprocal it lane-parallel out of PSUM,
                # multiply into O^T. All on-chip; no DRAM bounce.
                # den borrows the ps pool: its previous occupants' readers
                # are always already emitted, so the WAR wait can't deadlock
                # the PE queue (pso's would).
                rinv = rcp.tile([64, 2, CH], f32, name=f"ri{tag}", tag="ri")
                for hf in range(2):
                    den = ps.tile([64, CH], f32, name=f"dn{tag}_{hf}", tag="mm")
                    nc.tensor.matmul(
                        den[:],
                        ones_row[HD : HD + 1, :],
                        o_un[HD : HD + 1, hf, :],
                        start=True,
                        stop=True,
                    )
                    nc.vector.reciprocal_approx_fast(
                        out=rinv[:, hf, :], in_=den[:]
                    )
                nc.vector.tensor_mul(
                    out=o_sb[0:64, :],
                    in0=o_un[0:HD, 0, :],
                    in1=rinv[:, 0, :],
                )
                tmp = tmpb.tile([64, CH], bf16, name=f"tm{tag}", tag="tm")
                nc.vector.tensor_mul(
                    out=tmp[:], in0=o_un[0:HD, 1, :], in1=rinv[:, 1, :]
                )
                nc.sync.dma_start(out=o_sb[64:128, :], in_=tmp[:])

            def phase1_gen(j):
                """QKV projection for chunk j: 12 quanta of 8 matmuls each."""
                t0 = j * CH
                xt = xt_tiles[j]
                qt = qtp.tile([128, 4, CH], bf16, name=f"qt{j}", tag="qt")
                qt_tiles[j] = qt
                for m in range(8):
                    acc = ps.tile([128, CH], f32, name=f"qk_{j}_{m}", tag="mm")
                    for kt in range(8):
                        nc.tensor.matmul(
                            acc[:],
                            wqk_sb[:, kt, m * 128 : (m + 1) * 128],
                            xt[:, kt, :],
                            start=(kt == 0),
                            stop=(kt == 7),
                        )
                    if m < 4:
                        nc.vector.tensor_copy(out=qt[:, m, :], in_=acc[:])
                    else:
                        nc.vector.tensor_copy(
                            out=kt_sb[:, m % 4, t0 : t0 + CH], in_=acc[:]
                        )
                    yield
                for s in range(CH // 128):
                    accv = ps.tile([128, F], f32, name=f"v_{j}_{s}", tag="mm")
                    for kt in range(8):
                        nc.tensor.matmul(
                            accv[:],
                            xt[:, kt, s * 128 : (s + 1) * 128],
                            wv_sb[:, kt, :],
                            start=(kt == 0),
                            stop=(kt == 7),
                        )
                    nc.vector.tensor_copy(
                        out=v_sb[:, 4 * j + s, :, 0:HD],
                        in_=accv[:].rearrange("p (h d) -> p h d", h=HPC),
                    )
                    yield

            def phase3_gen(j):
                """Out-projection for chunk j: 8 quanta of 4 matmuls each."""
                q0 = j * CH
                for ot in range(8):
                    acc = ps.tile([128, CH], f32, name=f"y_{j}_{ot}", tag="mm")
                    for ft in range(4):
                        nc.tensor.matmul(
                            acc[:],
                            wo_sb[:, ft, ot * 128 : (ot + 1) * 128],
                            osb_tiles[j][ft][:, :],
                            start=(ft == 0),
                            stop=(ft == 3),
                        )
                    y = ysb.tile([128, CH], f32, name=f"ysb_{j}_{ot}", tag="y")
                    nc.vector.tensor_copy(out=y[:], in_=acc[:])
                    nc.sync.dma_start(
                        out=yT[ot * 128 : (ot + 1) * 128, q0 : q0 + CH], in_=y[:]
                    )
                    yield

            def phase2(j, filler=None, per_hp=1):
                q0 = j * CH
                nkt = 4 * j + 4
                osb_tiles[j] = []

                def pull(n):
                    if filler is None:
                        return
                    for _ in range(n):
                        if next(filler, None) is None:
                            return

                for hp in range(4):
                    o_sb = obuf.tile(
                        [128, CH], bf16, name=f"osb{j}_{hp}", tag="osb"
                    )
                    osb_tiles[j].append(o_sb)
                    o_un = oun.tile(
                        [HD + 1, 2, CH], f32r, name=f"oun_{j}_{hp}", tag="oun"
                    )
                    o_ps = [
                        pso.tile([HD + 1, CH], f32, name=f"o_{j}_{hp}_{hf}", tag="o")
                        for hf in range(2)
                    ]

                    def av(kt, p_t, lo):
                        for hf in range(2):
                            h = 2 * hp + hf
                            nc.tensor.matmul(
                                o_ps[hf][:, lo:CH],
                                v_sb[:, kt, h, :],
                                p_t[:, hf, lo:CH],
                                start=(kt == 0),
                                stop=(kt == nkt - 1),
                            )

                    prev = None
                    for kt in range(nkt):
                        k0 = kt * 128
                        lo = max(k0 - q0, 0)
                        # both heads' scores in one 2-bank PSUM tile so a
                        # single wide exp covers the pair; the two K=64
                        # matmuls sit on disjoint PE row groups (0-63 /
                        # 64-127) and run concurrently in the array.
                        s_t = pss.tile(
                            [128, 2, CH], f32, name=f"s_{j}_{hp}_{kt}", tag="s"
                        )
                        p_t = pexp.tile(
                            [128, 2, CH],
                            bf16,
                            name=f"p_{j}_{hp}_{kt}",
                            tag="p",
                        )
                        for hf in range(2):
                            pb = hf * 64
                            nc.tensor.matmul(
                                s_t[:, hf, lo:CH],
                                kt_sb[pb : pb + 64, hp, k0 : k0 + 128],
                                qt_tiles[j][pb : pb + 64, hp, lo:CH],
                                start=True,
                                stop=True,
                            )
                        nc.scalar.activation(
                            out=p_t[:, :, lo:CH],
                            in_=s_t[:, :, lo:CH],
                            func=mybir.ActivationFunctionType.Exp,
                            scale=0.125,
                        )
                        if k0 >= q0:
                            for hf in range(2):
                                nc.vector.tensor_mul(
                                    out=p_t[:, hf, lo : lo + 128],
                                    in0=p_t[:, hf, lo : lo + 128],
                                    in1=tri[:],
                                )
                        # AV runs one k-tile behind so exp(kt) overlaps it
                        if prev is not None:
                            av(*prev)
                        if kt == 1:
                            # previous head-pair's normalization: emitted here
                            # so its den matmul sits behind already-ready QK
                            # work in the PE queue instead of stalling it.
                            emit_norm()
                        if kt == nkt // 2:
                            pull(1)
                        prev = (kt, p_t, lo)
                    av(*prev)
                    for hf in range(2):
                        nc.vector.tensor_copy(
                            out=o_un[:, hf, :], in_=o_ps[hf][:, :]
                        )
                    pending_norm[0] = (o_un, o_sb, f"{j}_{hp}")
                    pull(per_hp)

            def drain(gen):
                for _ in gen:
                    pass

            # emission schedule: phase2's exp-paced stretches are stuffed
            # with projection quanta; each phase2(j) also flushes the
            # previous chunk's last normalization at its first head-pair.
            drain(phase1_gen(0))
            prefetch_xt(1)
            drain(phase1_gen(1))
            prefetch_xt(2)
            p1_2 = phase1_gen(2)
            phase2(0, filler=p1_2, per_hp=1)
            # wo is first needed by phase3(0); emitting its DMA here keeps
            # 1MB out of the startup queue
            nc.sync.dma_start(
                out=wo_sb[:], in_=wo.rearrange("(ft p) o -> p ft o", p=128)
            )
            drain(p1_2)
            prefetch_xt(3)
            p3_0 = phase3_gen(0)
            p1_3 = phase1_gen(3)
            phase2(1, filler=chain2(p3_0, p1_3), per_hp=2)
            drain(p3_0)
            drain(p1_3)
            p3_1 = phase3_gen(1)
            phase2(2, filler=p3_1, per_hp=2)
            drain(p3_1)
            p3_2 = phase3_gen(2)
            phase2(3, filler=p3_2, per_hp=2)
            drain(p3_2)
            emit_norm()
            drain(phase3_gen(3))

    nc.compile()
    return nc


def chain2(a, b):
    yield from a
    yield from b


def to_bf16(a: np.ndarray) -> np.ndarray:
    return np.ascontiguousarray(a, dtype=np.float32).astype(ml_dtypes.bfloat16)


def shard_inputs(x, W_qkv, W_out):
    """Build the 8 per-core input maps."""
    xT = [to_bf16(x[b].T) for b in range(B)]
    maps = []
    for core in range(NCORES):
        b, hf = core // 2, core % 2
        wq = W_qkv[:, hf * F : (hf + 1) * F]
        wk = W_qkv[:, C + hf * F : C + (hf + 1) * F]
        wv = W_qkv[:, 2 * C + hf * F : 2 * C + (hf + 1) * F]
        maps.append(
            {
                "xT": xT[b],
                "wqk": to_bf16(np.concatenate([wq, wk], axis=1)),
                "wv": to_bf16(wv),
                "wo": to_bf16(W_out[hf * F : (hf + 1) * F, :]),
            }
        )
    return maps


_NC_CACHE = {}


def get_nc():
    if "nc" not in _NC_CACHE:
        _NC_CACHE["nc"] = build_nc()
    return _NC_CACHE["nc"]


def kernel(x, W_qkv, W_out, _run_kwargs=None):
    x = np.asarray(x, dtype=np.float32)
    W_qkv = np.asarray(W_qkv, dtype=np.float32)
    W_out = np.asarray(W_out, dtype=np.float32)
    nc = get_nc()
    maps = shard_inputs(x, W_qkv, W_out)
    res = run_bass_kernel_spmd(nc, maps, list(range(NCORES)), **(_run_kwargs or {}))
    out = np.empty((B, T, C), dtype=np.float32)
    for b in range(B):
        yT0 = res.results[2 * b]["yT"]
        yT1 = res.results[2 * b + 1]["yT"]
        out[b] = (yT0 + yT1).T
    if _run_kwargs is not None:
        _NC_CACHE["last_results"] = res
    return out


# revision 23
# speedup vs baseline: 1.2726x; 1.0408x over previous
"""Causal self-attention (B=4, T=2048, C=1024, H=16) on 8 TRN2 NeuronCores.

Sharding: batch x head-halves. Core i handles batch b=i//2 and heads
[8*(i%2), 8*(i%2)+8). Each core computes QKV projection for its slice,
causal attention for its 8 heads, and a partial output projection
(512 of 1024 contraction features). The host sums the two partials per
batch and transposes back.

All matmul operands are bf16 (fp32 PSUM accumulation): 1 cycle/row on
the PE with no small-N penalty, and half the DMA/SBUF footprint of
fp32 -- the startup weight+activation load that gates the first real
matmuls drops from 6MB to 3MB, keeping the PE out of its cold
(1.2GHz) HAM state sooner.

Attention per head works on S^T tiles: S^T[k, q] so that the AV matmul
(lhsT = V [k, d+1], rhs = P^T [k, q]) needs no transposes, with a ones
column appended to V so row 64 of the PSUM accumulator collects the
softmax denominators. The two heads of a pair run their QK^T matmuls
concurrently in the PE array (disjoint 64-row groups). exp happens on
the scalar engine PSUM->SBUF with the 1/8 scale folded in; no max
subtraction (scores are ~N(0,1), safe in fp32). Causality: (k-tile,
q-chunk) pairs above the diagonal are skipped, diagonal tiles compute
only the valid columns and get a triangular mask multiply.

Softmax normalization stays on-chip: the denominator row of o_un is
broadcast down 64 partitions with a K=1 matmul against a ones row,
reciprocal'd lane-parallel (approx-fast, ~18 bits) straight out of
PSUM, and multiplied into O^T. No DRAM bounce. Each head-pair's
normalization is emitted one head-pair late so the tiny broadcast
matmul never parks at the head of the PE queue waiting on the o_un
copy.

Scheduling: the attention inner loop is exp-paced (the scalar engine
needs ~985ns per k-tile vs ~640ns of PE work), so projection work is
*woven into* phase2's emission stream as quanta -- one QKV-projection
m-block or one out-projection column tile at a time -- pulled mid
head-pair and after each head-pair. This keeps the PE queue stocked
with ready matmuls through the exp-paced stretches, which both fills
the idle cycles and stops the HAM clock gate from re-throttling the
array to 1.2GHz mid-kernel.
"""

import sys

if "/opt/trn_rl_repo" not in sys.path:
    sys.path.insert(0, "/opt/trn_rl_repo")

import numpy as np
import ml_dtypes

import concourse.bass as bass
import concourse.mybir as mybir
import concourse.tile as tile
from concourse import bacc
from concourse.bass_utils import run_bass_kernel_spmd
from concourse.masks import make_upper_triangular

B, T, C, H = 4, 2048, 1024, 16
HD = C // H  # 64
NCORES = 8
HPC = H // 2  # heads per core = 8
F = HPC * HD  # 512 features per core
CH = 512  # t/q chunk width
NCH = T // CH  # 4
NKT = T // 128  # 16 k-tiles

f32 = mybir.dt.float32
f32r = mybir.dt.float32r
bf16 = mybir.dt.bfloat16


def build_nc():
    nc = bacc.Bacc("TRN2", target_bir_lowering=False, debug=False)
    xT = nc.dram_tensor("xT", [C, T], bf16, kind="ExternalInput").ap()
    wqk = nc.dram_tensor("wqk", [C, 2 * F], bf16, kind="ExternalInput").ap()
    wv = nc.dram_tensor("wv", [C, F], bf16, kind="ExternalInput").ap()
    wo = nc.dram_tensor("wo", [F, C], bf16, kind="ExternalInput").ap()
    # y partials leave the core in bf16: the host sums the two partials per
    # batch in fp32; the extra ~0.4% partial quantization stays well inside
    # the error budget and halves the 8MB/core output DMA.
    yT = nc.dram_tensor("yT", [C, T], bf16, kind="ExternalOutput").ap()

    with tile.TileContext(nc) as tc:
        with (
            tc.tile_pool(name="consts", bufs=1) as consts,
            tc.tile_pool(name="kv", bufs=1) as kv,
            # all 4 qt tiles coexist: qt(j+2)'s projection copies are woven
            # into phase2(j)'s emission, so reusing qt(j)'s buffer would make
            # that copy wait on phase2(j) QK matmuls emitted *after* it -- a
            # cross-queue deadlock. 4 buffers removes the rotation entirely.
            tc.tile_pool(name="qtp", bufs=4) as qtp,
            tc.tile_pool(name="win", bufs=1) as win,
            tc.tile_pool(name="xin", bufs=2) as xin,
            tc.tile_pool(name="wout", bufs=1) as wout,
            tc.tile_pool(name="obuf", bufs=8) as obuf,
            tc.tile_pool(name="oun", bufs=2) as oun,
            tc.tile_pool(name="pexp", bufs=4) as pexp,
            tc.tile_pool(name="rcp", bufs=2) as rcp,
            tc.tile_pool(name="tmpb", bufs=2) as tmpb,
            tc.tile_pool(name="ysb", bufs=2) as ysb,
            tc.tile_pool(name="ypp", bufs=8) as ypp,
            tc.tile_pool(name="ps", bufs=2, space="PSUM") as ps,
            tc.tile_pool(name="pss", bufs=2, space="PSUM") as pss,
            tc.tile_pool(name="pso", bufs=2, space="PSUM") as pso,
        ):
            tri = consts.tile([128, 128], f32)
            make_upper_triangular(nc, tri[:], val=1.0, diag=True)

            kt_sb = kv.tile([128, 4, T], bf16)  # K^T; head 2p|2p+1 on parts 0-63|64-127
            # V aug ([t, head, d] + ones col): stationary operand of the
            # AV matmul; bf16 error passes linearly into O (~0.2% rel).
            v_sb = kv.tile([128, NKT, HPC, HD + 1], bf16)
            ones = consts.tile([128, NKT * HPC], f32)
            nc.vector.memset(ones[:], 1.0)
            nc.vector.tensor_copy(
                out=v_sb[:, :, :, HD : HD + 1],
                in_=ones[:].rearrange("p (a b c) -> p a b c", a=NKT, b=HPC),
            )
            # ones row for the denominator broadcast matmul (K=1, M=64).
            # [65, 64] so row 64 can be sliced: the matmul's lhsT must sit on
            # the same base partition as the denominator row of o_un (= HD).
            ones_row = consts.tile([HD + 1, 64], f32r)
            nc.vector.tensor_copy(out=ones_row[:], in_=ones[0 : HD + 1, 0:64])

            # ~7us of dummy matmuls at t=0: spins the PE through the HAM
            # activity window while the first input DMAs land, so phase1
            # starts at 2.4GHz instead of ramping from the 1.2GHz cold state.
            warm = consts.tile([128, CH], bf16)
            nc.vector.memset(warm[:], 1.0)
            wps = ps.tile([128, CH], f32, name="warmps", tag="mm")
            for i in range(16):
                nc.tensor.matmul(
                    wps[:], warm[:, 0:128], warm[:], start=True, stop=True
                )

            # weights: split + interleave DMAs per k-tile so the first
            # matmul's operands (wqk[0], xt[0]) land within ~1us
            xt_tiles = {}
            xt_tiles[0] = xin.tile([128, 8, CH], bf16, name="xt0", tag="xt")
            wqk_sb = win.tile([128, 8, 2 * F], bf16)
            wv_sb = win.tile([128, 8, F], bf16)
            for kt in range(8):
                nc.sync.dma_start(
                    out=wqk_sb[:, kt, :], in_=wqk[kt * 128 : (kt + 1) * 128, :]
                )
                nc.sync.dma_start(
                    out=xt_tiles[0][:, kt, :],
                    in_=xT[kt * 128 : (kt + 1) * 128, 0:CH],
                )
            for kt in range(8):
                nc.sync.dma_start(
                    out=wv_sb[:, kt, :], in_=wv[kt * 128 : (kt + 1) * 128, :]
                )
            wo_sb = wout.tile([128, 4, C], bf16)

            def prefetch_xt(j):
                # emitted well before phase1(j)'s quanta so the DMA lands
                # before any matmul in the FIFO waits on it
                xt = xin.tile([128, 8, CH], bf16, name=f"xt{j}", tag="xt")
                xt_tiles[j] = xt
                t0 = j * CH
                nc.sync.dma_start(
                    out=xt[:],
                    in_=xT[:, t0 : t0 + CH].rearrange("(kt p) t -> p kt t", p=128),
                )

            qt_tiles = {}
            osb_tiles = {}
            pending_norm = [None]

            def emit_norm():
                if pending_norm[0] is None:
                    return
                o_un, o_sb, tag = pending_norm[0]
                pending_norm[0] = None
                # broadcast the denominator row down 64 partitions with a
                # K=1 matmul, reciprocal it lane-parallel out of PSUM,
                # multiply into O^T. All on-chip; no DRAM bounce.
                # den borrows the ps pool: its previous occupants' readers
                # are always already emitted, so the WAR wait can't deadlock
                # the PE queue (pso's would).
                rinv = rcp.tile([64, 2, CH], f32, name=f"ri{tag}", tag="ri")
                for hf in range(2):
                    den = ps.tile([64, CH], f32, name=f"dn{tag}_{hf}", tag="mm")
                    nc.tensor.matmul(
                        den[:],
                        ones_row[HD : HD + 1, :],
                        o_un[HD : HD + 1, hf, :],
                        start=True,
                        stop=True,
                    )
                    nc.vector.reciprocal_approx_fast(
                        out=rinv[:, hf, :], in_=den[:]
                    )
                nc.vector.tensor_mul(
                    out=o_sb[0:64, :],
                    in0=o_un[0:HD, 0, :],
                    in1=rinv[:, 0, :],
                )
                tmp = tmpb.tile([64, CH], bf16, name=f"tm{tag}", tag="tm")
                nc.vector.tensor_mul(
                    out=tmp[:], in0=o_un[0:HD, 1, :], in1=rinv[:, 1, :]
                )
                nc.sync.dma_start(out=o_sb[64:128, :], in_=tmp[:])

            def phase1_gen(j):
                """QKV projection for chunk j: 12 quanta of 8 matmuls each."""
                t0 = j * CH
                xt = xt_tiles[j]
                qt = qtp.tile([128, 4, CH], bf16, name=f"qt{j}", tag="qt")
                qt_tiles[j] = qt
                for m in range(8):
                    acc = ps.tile([128, CH], f32, name=f"qk_{j}_{m}", tag="mm")
                    for kt in range(8):
                        nc.tensor.matmul(
                            acc[:],
                            wqk_sb[:, kt, m * 128 : (m + 1) * 128],
                            xt[:, kt, :],
                            start=(kt == 0),
                            stop=(kt == 7),
                        )
                    if m < 4:
                        nc.vector.tensor_copy(out=qt[:, m, :], in_=acc[:])
                    else:
                        nc.vector.tensor_copy(
                            out=kt_sb[:, m % 4, t0 : t0 + CH], in_=acc[:]
                        )
                    yield
                for s in range(CH // 128):
                    accv = ps.tile([128, F], f32, name=f"v_{j}_{s}", tag="mm")
                    for kt in range(8):
                        nc.tensor.matmul(
                            accv[:],
                            xt[:, kt, s * 128 : (s + 1) * 128],
                            wv_sb[:, kt, :],
                            start=(kt == 0),
                            stop=(kt == 7),
                        )
                    nc.vector.tensor_copy(
                        out=v_sb[:, 4 * j + s, :, 0:HD],
                        in_=accv[:].rearrange("p (h d) -> p h d", h=HPC),
                    )
                    yield

            def phase3_gen(j):
                """Out-projection for chunk j: 8 quanta of 4 matmuls each."""
                q0 = j * CH
                for ot in range(8):
                    acc = ps.tile([128, CH], f32, name=f"y_{j}_{ot}", tag="mm")
                    for ft in range(4):
                        nc.tensor.matmul(
                            acc[:],
                            wo_sb[:, ft, ot * 128 : (ot + 1) * 128],
                            osb_tiles[j][ft][:, :],
                            start=(ft == 0),
                            stop=(ft == 3),
                        )
                    y = ysb.tile([128, CH], bf16, name=f"ysb_{j}_{ot}", tag="y")
                    nc.vector.tensor_copy(out=y[:], in_=acc[:])
                    nc.sync.dma_start(
                        out=yT[ot * 128 : (ot + 1) * 128, q0 : q0 + CH], in_=y[:]
                    )
                    yield

            ypart = {}

            def phase3_partial_gen(j):
                """First 3 of 4 contraction tiles of chunk j's out-projection,
                copied to SBUF partials. Weavable as soon as head-pairs 0-2
                are normalized -- i.e. into phase2(j)'s own last head-pair --
                which is exactly where the exp-paced deficit is largest and
                no other filler is legal."""
                for ot in range(8):
                    acc = ps.tile([128, CH], f32, name=f"yp_{j}_{ot}", tag="mm")
                    for ft in range(3):
                        nc.tensor.matmul(
                            acc[:],
                            wo_sb[:, ft, ot * 128 : (ot + 1) * 128],
                            osb_tiles[j][ft][:, :],
                            start=(ft == 0),
                            stop=(ft == 2),
                        )
                    yp = ypp.tile([128, CH], bf16, name=f"ypp_{j}_{ot}", tag="yp")
                    ypart[(j, ot)] = yp
                    nc.vector.tensor_copy(out=yp[:], in_=acc[:])
                    yield

            def phase3_final(j):
                """Remaining contraction tile (the last head-pair) + add."""
                q0 = j * CH
                for ot in range(8):
                    acc = ps.tile([128, CH], f32, name=f"yf_{j}_{ot}", tag="mm")
                    nc.tensor.matmul(
                        acc[:],
                        wo_sb[:, 3, ot * 128 : (ot + 1) * 128],
                        osb_tiles[j][3][:, :],
                        start=True,
                        stop=True,
                    )
                    y = ysb.tile([128, CH], bf16, name=f"ysf_{j}_{ot}", tag="y")
                    nc.vector.tensor_add(
                        out=y[:], in0=ypart[(j, ot)][:], in1=acc[:]
                    )
                    nc.sync.dma_start(
                        out=yT[ot * 128 : (ot + 1) * 128, q0 : q0 + CH], in_=y[:]
                    )

            def phase2(j, filler=None, late=None, per_hp=1):
                q0 = j * CH
                nkt = 4 * j + 4
                osb_tiles[j] = []

                def pull(n, src):
                    if src is None:
                        return
                    for _ in range(n):
                        if next(src, None) is None:
                            return

                for hp in range(4):
                    o_sb = obuf.tile(
                        [128, CH], bf16, name=f"osb{j}_{hp}", tag="osb"
                    )
                    osb_tiles[j].append(o_sb)
                    o_un = oun.tile(
                        [HD + 1, 2, CH], f32r, name=f"oun_{j}_{hp}", tag="oun"
                    )
                    o_ps = [
                        pso.tile([HD + 1, CH], f32, name=f"o_{j}_{hp}_{hf}", tag="o")
                        for hf in range(2)
                    ]

                    def av(kt, p_t, lo):
                        for hf in range(2):
                            h = 2 * hp + hf
                            nc.tensor.matmul(
                                o_ps[hf][:, lo:CH],
                                v_sb[:, kt, h, :],
                                p_t[:, hf, lo:CH],
                                start=(kt == 0),
                                stop=(kt == nkt - 1),
                            )

                    prev = None
                    for kt in range(nkt):
                        k0 = kt * 128
                        lo = max(k0 - q0, 0)
                        # both heads' scores in one 2-bank PSUM tile so a
                        # single wide exp covers the pair; the two K=64
                        # matmuls sit on disjoint PE row groups (0-63 /
                        # 64-127) and run concurrently in the array.
                        s_t = pss.tile(
                            [128, 2, CH], f32, name=f"s_{j}_{hp}_{kt}", tag="s"
                        )
                        p_t = pexp.tile(
                            [128, 2, CH],
                            bf16,
                            name=f"p_{j}_{hp}_{kt}",
                            tag="p",
                        )
                        for hf in range(2):
                            pb = hf * 64
                            nc.tensor.matmul(
                                s_t[:, hf, lo:CH],
                                kt_sb[pb : pb + 64, hp, k0 : k0 + 128],
                                qt_tiles[j][pb : pb + 64, hp, lo:CH],
                                start=True,
                                stop=True,
                            )
                        nc.scalar.activation(
                            out=p_t[:, :, lo:CH],
                            in_=s_t[:, :, lo:CH],
                            func=mybir.ActivationFunctionType.Exp,
                            scale=0.125,
                        )
                        if k0 >= q0:
                            for hf in range(2):
                                nc.vector.tensor_mul(
                                    out=p_t[:, hf, lo : lo + 128],
                                    in0=p_t[:, hf, lo : lo + 128],
                                    in1=tri[:],
                                )
                        # AV runs one k-tile behind so exp(kt) overlaps it
                        if prev is not None:
                            av(*prev)
                        if kt == 1:
                            # previous head-pair's normalization: emitted here
                            # so its den matmul sits behind already-ready QK
                            # work in the PE queue instead of stalling it.
                            emit_norm()
                        if kt % 4 == 3:
                            # the attention inner loop runs a ~345ns/k-tile
                            # PE deficit against the exp pacer; one filler
                            # quantum per 4 k-tiles roughly balances it.
                            pull(1, late if hp == 3 and late is not None else filler)
                        prev = (kt, p_t, lo)
                    av(*prev)
                    for hf in range(2):
                        nc.vector.tensor_copy(
                            out=o_un[:, hf, :], in_=o_ps[hf][:, :]
                        )
                    pending_norm[0] = (o_un, o_sb, f"{j}_{hp}")
                    pull(per_hp, late if hp == 3 and late is not None else filler)

            def drain(gen):
                for _ in gen:
                    pass

            # emission schedule: phase2's exp-paced stretches are stuffed
            # with projection quanta; each phase2(j) also flushes the
            # previous chunk's last normalization at its first head-pair.
            # Filler budget per phase2 window ~= its exp-pacing deficit:
            # ph2(0) 5.5us <- ph1(2); ph2(1) 11us <- ph3(0)+ph1(2) rest;
            # ph2(2) 16.6us <- ph1(3); ph2(3) 16.5+5.5us <- ph3(1)+ph3(2)
            # + ph3(3) ft0-2 partials (the only work legal in its last
            # head-pair). Leftovers drain between windows; every drain-point
            # also enforces a must-finish-before ordering constraint.
            drain(phase1_gen(0))
            prefetch_xt(1)
            drain(phase1_gen(1))
            prefetch_xt(2)
            p1_2 = phase1_gen(2)
            phase2(0, filler=p1_2, per_hp=1)
            # wo is first needed by phase3(0); emitting its DMA here keeps
            # 1MB out of the startup queue
            nc.sync.dma_start(
                out=wo_sb[:], in_=wo.rearrange("(ft p) o -> p ft o", p=128)
            )
            prefetch_xt(3)
            p3_0 = phase3_gen(0)
            p1_3 = phase1_gen(3)
            phase2(1, filler=chain3(p3_0, p1_2, p1_3), per_hp=1)
            drain(p1_2)
            drain(p3_0)
            phase2(2, filler=p1_3, per_hp=2)
            drain(p1_3)
            p3_1 = phase3_gen(1)
            p3_2 = phase3_gen(2)
            phase2(3, filler=chain2(p3_1, p3_2), per_hp=2)
            drain(p3_1)
            drain(p3_2)
            emit_norm()
            drain(phase3_gen(3))

    nc.compile()
    return nc


def chain2(a, b):
    yield from a
    yield from b


def chain3(a, b, c):
    yield from a
    yield from b
    yield from c


def to_bf16(a: np.ndarray) -> np.ndarray:
    return np.ascontiguousarray(a, dtype=np.float32).astype(ml_dtypes.bfloat16)


def shard_inputs(x, W_qkv, W_out):
    """Build the 8 per-core input maps."""
    xT = [to_bf16(x[b].T) for b in range(B)]
    maps = []
    for core in range(NCORES):
        b, hf = core // 2, core % 2
        wq = W_qkv[:, hf * F : (hf + 1) * F]
        wk = W_qkv[:, C + hf * F : C + (hf + 1) * F]
        wv = W_qkv[:, 2 * C + hf * F : 2 * C + (hf + 1) * F]
        maps.append(
            {
                "xT": xT[b],
                "wqk": to_bf16(np.concatenate([wq, wk], axis=1)),
                "wv": to_bf16(wv),
                "wo": to_bf16(W_out[hf * F : (hf + 1) * F, :]),
            }
        )
    return maps


_NC_CACHE = {}


def get_nc():
    if "nc" not in _NC_CACHE:
        _NC_CACHE["nc"] = build_nc()
    return _NC_CACHE["nc"]


def kernel(x, W_qkv, W_out, _run_kwargs=None):
    x = np.asarray(x, dtype=np.float32)
    W_qkv = np.asarray(W_qkv, dtype=np.float32)
    W_out = np.asarray(W_out, dtype=np.float32)
    nc = get_nc()
    maps = shard_inputs(x, W_qkv, W_out)
    res = run_bass_kernel_spmd(nc, maps, list(range(NCORES)), **(_run_kwargs or {}))
    out = np.empty((B, T, C), dtype=np.float32)
    for b in range(B):
        yT0 = np.asarray(res.results[2 * b]["yT"], dtype=np.float32)
        yT1 = np.asarray(res.results[2 * b + 1]["yT"], dtype=np.float32)
        out[b] = (yT0 + yT1).T
    if _run_kwargs is not None:
        _NC_CACHE["last_results"] = res
    return out


# revision 24
# speedup vs baseline: 1.2726x; 1.0000x over previous
"""Causal self-attention (B=4, T=2048, C=1024, H=16) on 8 TRN2 NeuronCores.

Sharding: batch x head-halves. Core i handles batch b=i//2 and heads
[8*(i%2), 8*(i%2)+8). Each core computes QKV projection for its slice,
causal attention for its 8 heads, and a partial output projection
(512 of 1024 contraction features). The host sums the two partials per
batch and transposes back.

All matmul operands are bf16 (fp32 PSUM accumulation): 1 cycle/row on
the PE with no small-N penalty, and half the DMA/SBUF footprint of
fp32 -- the startup weight+activation load that gates the first real
matmuls drops from 6MB to 3MB, keeping the PE out of its cold
(1.2GHz) HAM state sooner.

Attention per head works on S^T tiles: S^T[k, q] so that the AV matmul
(lhsT = V [k, d+1], rhs = P^T [k, q]) needs no transposes, with a ones
column appended to V so row 64 of the PSUM accumulator collects the
softmax denominators. The two heads of a pair run their QK^T matmuls
concurrently in the PE array (disjoint 64-row groups). exp happens on
the scalar engine PSUM->SBUF with the 1/8 scale folded in; no max
subtraction (scores are ~N(0,1), safe in fp32). Causality: (k-tile,
q-chunk) pairs above the diagonal are skipped, diagonal tiles compute
only the valid columns and get a triangular mask multiply.

Softmax normalization stays on-chip: the denominator row of o_un is
broadcast down 64 partitions with a K=1 matmul against a ones row,
reciprocal'd lane-parallel (approx-fast, ~18 bits) straight out of
PSUM, and multiplied into O^T. No DRAM bounce. Each head-pair's
normalization is emitted one head-pair late so the tiny broadcast
matmul never parks at the head of the PE queue waiting on the o_un
copy.

Scheduling: the attention inner loop is exp-paced (the scalar engine
needs ~985ns per k-tile vs ~640ns of PE work), so projection work is
*woven into* phase2's emission stream as quanta -- one QKV-projection
m-block or one out-projection column tile at a time -- pulled mid
head-pair and after each head-pair. This keeps the PE queue stocked
with ready matmuls through the exp-paced stretches, which both fills
the idle cycles and stops the HAM clock gate from re-throttling the
array to 1.2GHz mid-kernel.
"""

import sys

if "/opt/trn_rl_repo" not in sys.path:
    sys.path.insert(0, "/opt/trn_rl_repo")

import numpy as np
import ml_dtypes

import concourse.bass as bass
import concourse.mybir as mybir
import concourse.tile as tile
from concourse import bacc
from concourse.bass_utils import run_bass_kernel_spmd
from concourse.masks import make_upper_triangular

B, T, C, H = 4, 2048, 1024, 16
HD = C // H  # 64
NCORES = 8
HPC = H // 2  # heads per core = 8
F = HPC * HD  # 512 features per core
CH = 512  # t/q chunk width
NCH = T // CH  # 4
NKT = T // 128  # 16 k-tiles

f32 = mybir.dt.float32
f32r = mybir.dt.float32r
bf16 = mybir.dt.bfloat16


def build_nc():
    nc = bacc.Bacc("TRN2", target_bir_lowering=False, debug=False)
    xT = nc.dram_tensor("xT", [C, T], bf16, kind="ExternalInput").ap()
    wqk = nc.dram_tensor("wqk", [C, 2 * F], bf16, kind="ExternalInput").ap()
    wv = nc.dram_tensor("wv", [C, F], bf16, kind="ExternalInput").ap()
    wo = nc.dram_tensor("wo", [F, C], bf16, kind="ExternalInput").ap()
    # y partials leave the core in bf16: the host sums the two partials per
    # batch in fp32; the extra ~0.4% partial quantization stays well inside
    # the error budget and halves the 8MB/core output DMA.
    yT = nc.dram_tensor("yT", [C, T], bf16, kind="ExternalOutput").ap()

    with tile.TileContext(nc) as tc:
        with (
            tc.tile_pool(name="consts", bufs=1) as consts,
            tc.tile_pool(name="kv", bufs=1) as kv,
            # all 4 qt tiles coexist: qt(j+2)'s projection copies are woven
            # into phase2(j)'s emission, so reusing qt(j)'s buffer would make
            # that copy wait on phase2(j) QK matmuls emitted *after* it -- a
            # cross-queue deadlock. 4 buffers removes the rotation entirely.
            tc.tile_pool(name="qtp", bufs=4) as qtp,
            tc.tile_pool(name="win", bufs=1) as win,
            tc.tile_pool(name="xin", bufs=2) as xin,
            tc.tile_pool(name="wout", bufs=1) as wout,
            tc.tile_pool(name="obuf", bufs=8) as obuf,
            tc.tile_pool(name="oun", bufs=2) as oun,
            tc.tile_pool(name="pexp", bufs=4) as pexp,
            tc.tile_pool(name="rcp", bufs=2) as rcp,
            tc.tile_pool(name="tmpb", bufs=2) as tmpb,
            tc.tile_pool(name="ysb", bufs=2) as ysb,
            tc.tile_pool(name="ypp", bufs=8) as ypp,
            tc.tile_pool(name="ps", bufs=2, space="PSUM") as ps,
            tc.tile_pool(name="pss", bufs=2, space="PSUM") as pss,
            tc.tile_pool(name="pso", bufs=2, space="PSUM") as pso,
        ):
            # ~7us of dummy matmuls FIRST: emitted before every other
            # constant so the PE's first instruction issues as soon as the
            # engines boot (~1.6us), spinning the array through the HAM
            # activity window while the input DMAs land. phase1 then starts
            # at 2.4GHz instead of ramping from the 1.2GHz cold state.
            warm = consts.tile([128, CH], bf16)
            nc.vector.memset(warm[:], 1.0)
            wps = ps.tile([128, CH], f32, name="warmps", tag="mm")
            for i in range(16):
                nc.tensor.matmul(
                    wps[:], warm[:, 0:128], warm[:], start=True, stop=True
                )

            # weights: split + interleave DMAs per k-tile so the first
            # matmul's operands (wqk[0], xt[0]) land within ~1us
            xt_tiles = {}
            xt_tiles[0] = xin.tile([128, 8, CH], bf16, name="xt0", tag="xt")
            wqk_sb = win.tile([128, 8, 2 * F], bf16)
            wv_sb = win.tile([128, 8, F], bf16)
            for kt in range(8):
                nc.sync.dma_start(
                    out=wqk_sb[:, kt, :], in_=wqk[kt * 128 : (kt + 1) * 128, :]
                )
                nc.sync.dma_start(
                    out=xt_tiles[0][:, kt, :],
                    in_=xT[kt * 128 : (kt + 1) * 128, 0:CH],
                )
            for kt in range(8):
                nc.sync.dma_start(
                    out=wv_sb[:, kt, :], in_=wv[kt * 128 : (kt + 1) * 128, :]
                )
            wo_sb = wout.tile([128, 4, C], bf16)

            tri = consts.tile([128, 128], f32)
            make_upper_triangular(nc, tri[:], val=1.0, diag=True)

            kt_sb = kv.tile([128, 4, T], bf16)  # K^T; head 2p|2p+1 on parts 0-63|64-127
            # V aug ([t, head, d] + ones col): stationary operand of the
            # AV matmul; bf16 error passes linearly into O (~0.2% rel).
            v_sb = kv.tile([128, NKT, HPC, HD + 1], bf16)
            ones = consts.tile([128, NKT * HPC], f32)
            nc.vector.memset(ones[:], 1.0)
            nc.vector.tensor_copy(
                out=v_sb[:, :, :, HD : HD + 1],
                in_=ones[:].rearrange("p (a b c) -> p a b c", a=NKT, b=HPC),
            )
            # ones row for the denominator broadcast matmul (K=1, M=64).
            # [65, 64] so row 64 can be sliced: the matmul's lhsT must sit on
            # the same base partition as the denominator row of o_un (= HD).
            ones_row = consts.tile([HD + 1, 64], f32r)
            nc.vector.tensor_copy(out=ones_row[:], in_=ones[0 : HD + 1, 0:64])

            def prefetch_xt(j):
                # emitted well before phase1(j)'s quanta so the DMA lands
                # before any matmul in the FIFO waits on it
                xt = xin.tile([128, 8, CH], bf16, name=f"xt{j}", tag="xt")
                xt_tiles[j] = xt
                t0 = j * CH
                nc.sync.dma_start(
                    out=xt[:],
                    in_=xT[:, t0 : t0 + CH].rearrange("(kt p) t -> p kt t", p=128),
                )

            qt_tiles = {}
            osb_tiles = {}
            pending_norm = [None]

            def emit_norm():
                if pending_norm[0] is None:
                    return
                o_un, o_sb, tag = pending_norm[0]
                pending_norm[0] = None
                # broadcast the denominator row down 64 partitions with a
                # K=1 matmul, reciprocal it lane-parallel out of PSUM,
                # multiply into O^T. All on-chip; no DRAM bounce.
                # den borrows the ps pool: its previous occupants' readers
                # are always already emitted, so the WAR wait can't deadlock
                # the PE queue (pso's would).
                rinv = rcp.tile([64, 2, CH], f32, name=f"ri{tag}", tag="ri")
                for hf in range(2):
                    den = ps.tile([64, CH], f32, name=f"dn{tag}_{hf}", tag="mm")
                    nc.tensor.matmul(
                        den[:],
                        ones_row[HD : HD + 1, :],
                        o_un[HD : HD + 1, hf, :],
                        start=True,
                        stop=True,
                    )
                    nc.vector.reciprocal_approx_fast(
                        out=rinv[:, hf, :], in_=den[:]
                    )
                nc.vector.tensor_mul(
                    out=o_sb[0:64, :],
                    in0=o_un[0:HD, 0, :],
                    in1=rinv[:, 0, :],
                )
                tmp = tmpb.tile([64, CH], bf16, name=f"tm{tag}", tag="tm")
                nc.vector.tensor_mul(
                    out=tmp[:], in0=o_un[0:HD, 1, :], in1=rinv[:, 1, :]
                )
                nc.sync.dma_start(out=o_sb[64:128, :], in_=tmp[:])

            def phase1_gen(j):
                """QKV projection for chunk j: 12 quanta of 8 matmuls each."""
                t0 = j * CH
                xt = xt_tiles[j]
                qt = qtp.tile([128, 4, CH], bf16, name=f"qt{j}", tag="qt")
                qt_tiles[j] = qt
                for m in range(8):
                    acc = ps.tile([128, CH], f32, name=f"qk_{j}_{m}", tag="mm")
                    for kt in range(8):
                        nc.tensor.matmul(
                            acc[:],
                            wqk_sb[:, kt, m * 128 : (m + 1) * 128],
                            xt[:, kt, :],
                            start=(kt == 0),
                            stop=(kt == 7),
                        )
                    if m < 4:
                        nc.vector.tensor_copy(out=qt[:, m, :], in_=acc[:])
                    else:
                        nc.vector.tensor_copy(
                            out=kt_sb[:, m % 4, t0 : t0 + CH], in_=acc[:]
                        )
                    yield
                for s in range(CH // 128):
                    accv = ps.tile([128, F], f32, name=f"v_{j}_{s}", tag="mm")
                    for kt in range(8):
                        nc.tensor.matmul(
                            accv[:],
                            xt[:, kt, s * 128 : (s + 1) * 128],
                            wv_sb[:, kt, :],
                            start=(kt == 0),
                            stop=(kt == 7),
                        )
                    nc.vector.tensor_copy(
                        out=v_sb[:, 4 * j + s, :, 0:HD],
                        in_=accv[:].rearrange("p (h d) -> p h d", h=HPC),
                    )
                    yield

            def phase3_gen(j):
                """Out-projection for chunk j: 8 quanta of 4 matmuls each."""
                q0 = j * CH
                for ot in range(8):
                    acc = ps.tile([128, CH], f32, name=f"y_{j}_{ot}", tag="mm")
                    for ft in range(4):
                        nc.tensor.matmul(
                            acc[:],
                            wo_sb[:, ft, ot * 128 : (ot + 1) * 128],
                            osb_tiles[j][ft][:, :],
                            start=(ft == 0),
                            stop=(ft == 3),
                        )
                    y = ysb.tile([128, CH], bf16, name=f"ysb_{j}_{ot}", tag="y")
                    nc.vector.tensor_copy(out=y[:], in_=acc[:])
                    nc.sync.dma_start(
                        out=yT[ot * 128 : (ot + 1) * 128, q0 : q0 + CH], in_=y[:]
                    )
                    yield

            ypart = {}

            def phase3_partial_gen(j):
                """First 3 of 4 contraction tiles of chunk j's out-projection,
                copied to SBUF partials. Weavable as soon as head-pairs 0-2
                are normalized -- i.e. into phase2(j)'s own last head-pair --
                which is exactly where the exp-paced deficit is largest and
                no other filler is legal."""
                for ot in range(8):
                    acc = ps.tile([128, CH], f32, name=f"yp_{j}_{ot}", tag="mm")
                    for ft in range(3):
                        nc.tensor.matmul(
                            acc[:],
                            wo_sb[:, ft, ot * 128 : (ot + 1) * 128],
                            osb_tiles[j][ft][:, :],
                            start=(ft == 0),
                            stop=(ft == 2),
                        )
                    yp = ypp.tile([128, CH], bf16, name=f"ypp_{j}_{ot}", tag="yp")
                    ypart[(j, ot)] = yp
                    nc.vector.tensor_copy(out=yp[:], in_=acc[:])
                    yield

            def phase3_final(j):
                """Remaining contraction tile (the last head-pair) + add."""
                q0 = j * CH
                for ot in range(8):
                    acc = ps.tile([128, CH], f32, name=f"yf_{j}_{ot}", tag="mm")
                    nc.tensor.matmul(
                        acc[:],
                        wo_sb[:, 3, ot * 128 : (ot + 1) * 128],
                        osb_tiles[j][3][:, :],
                        start=True,
                        stop=True,
                    )
                    y = ysb.tile([128, CH], bf16, name=f"ysf_{j}_{ot}", tag="y")
                    nc.vector.tensor_add(
                        out=y[:], in0=ypart[(j, ot)][:], in1=acc[:]
                    )
                    nc.sync.dma_start(
                        out=yT[ot * 128 : (ot + 1) * 128, q0 : q0 + CH], in_=y[:]
                    )

            def phase2(j, filler=None, late=None, per_hp=1):
                q0 = j * CH
                nkt = 4 * j + 4
                osb_tiles[j] = []

                def pull(n, src):
                    if src is None:
                        return
                    for _ in range(n):
                        if next(src, None) is None:
                            return

                for hp in range(4):
                    o_sb = obuf.tile(
                        [128, CH], bf16, name=f"osb{j}_{hp}", tag="osb"
                    )
                    osb_tiles[j].append(o_sb)
                    o_un = oun.tile(
                        [HD + 1, 2, CH], f32r, name=f"oun_{j}_{hp}", tag="oun"
                    )
                    o_ps = [
                        pso.tile([HD + 1, CH], f32, name=f"o_{j}_{hp}_{hf}", tag="o")
                        for hf in range(2)
                    ]

                    def av(kt, p_t, lo):
                        for hf in range(2):
                            h = 2 * hp + hf
                            nc.tensor.matmul(
                                o_ps[hf][:, lo:CH],
                                v_sb[:, kt, h, :],
                                p_t[:, hf, lo:CH],
                                start=(kt == 0),
                                stop=(kt == nkt - 1),
                            )

                    prev = None
                    for kt in range(nkt):
                        k0 = kt * 128
                        lo = max(k0 - q0, 0)
                        # both heads' scores in one 2-bank PSUM tile so a
                        # single wide exp covers the pair; the two K=64
                        # matmuls sit on disjoint PE row groups (0-63 /
                        # 64-127) and run concurrently in the array.
                        s_t = pss.tile(
                            [128, 2, CH], f32, name=f"s_{j}_{hp}_{kt}", tag="s"
                        )
                        p_t = pexp.tile(
                            [128, 2, CH],
                            bf16,
                            name=f"p_{j}_{hp}_{kt}",
                            tag="p",
                        )
                        for hf in range(2):
                            pb = hf * 64
                            nc.tensor.matmul(
                                s_t[:, hf, lo:CH],
                                kt_sb[pb : pb + 64, hp, k0 : k0 + 128],
                                qt_tiles[j][pb : pb + 64, hp, lo:CH],
                                start=True,
                                stop=True,
                            )
                        nc.scalar.activation(
                            out=p_t[:, :, lo:CH],
                            in_=s_t[:, :, lo:CH],
                            func=mybir.ActivationFunctionType.Exp,
                            scale=0.125,
                        )
                        if k0 >= q0:
                            for hf in range(2):
                                nc.vector.tensor_mul(
                                    out=p_t[:, hf, lo : lo + 128],
                                    in0=p_t[:, hf, lo : lo + 128],
                                    in1=tri[:],
                                )
                        # AV runs one k-tile behind so exp(kt) overlaps it
                        if prev is not None:
                            av(*prev)
                        if kt == 1:
                            # previous head-pair's normalization: emitted here
                            # so its den matmul sits behind already-ready QK
                            # work in the PE queue instead of stalling it.
                            emit_norm()
                        if kt % 4 == 3:
                            # the attention inner loop runs a ~345ns/k-tile
                            # PE deficit against the exp pacer; one filler
                            # quantum per 4 k-tiles roughly balances it.
                            pull(1, late if hp == 3 and late is not None else filler)
                        prev = (kt, p_t, lo)
                    av(*prev)
                    for hf in range(2):
                        nc.vector.tensor_copy(
                            out=o_un[:, hf, :], in_=o_ps[hf][:, :]
                        )
                    pending_norm[0] = (o_un, o_sb, f"{j}_{hp}")
                    pull(per_hp, late if hp == 3 and late is not None else filler)

            def drain(gen):
                for _ in gen:
                    pass

            # emission schedule: phase2's exp-paced stretches are stuffed
            # with projection quanta; each phase2(j) also flushes the
            # previous chunk's last normalization at its first head-pair.
            # Filler budget per phase2 window ~= its exp-pacing deficit:
            # ph2(0) 5.5us <- ph1(2); ph2(1) 11us <- ph3(0)+ph1(2) rest;
            # ph2(2) 16.6us <- ph1(3); ph2(3) 16.5+5.5us <- ph3(1)+ph3(2)
            # + ph3(3) ft0-2 partials (the only work legal in its last
            # head-pair). Leftovers drain between windows; every drain-point
            # also enforces a must-finish-before ordering constraint.
            drain(phase1_gen(0))
            prefetch_xt(1)
            drain(phase1_gen(1))
            prefetch_xt(2)
            p1_2 = phase1_gen(2)
            phase2(0, filler=p1_2, per_hp=1)
            # wo is first needed by phase3(0); emitting its DMA here keeps
            # 1MB out of the startup queue
            nc.sync.dma_start(
                out=wo_sb[:], in_=wo.rearrange("(ft p) o -> p ft o", p=128)
            )
            prefetch_xt(3)
            p3_0 = phase3_gen(0)
            p1_3 = phase1_gen(3)
            phase2(1, filler=chain3(p3_0, p1_2, p1_3), per_hp=1)
            drain(p1_2)
            drain(p3_0)
            phase2(2, filler=p1_3, per_hp=2)
            drain(p1_3)
            p3_1 = phase3_gen(1)
            p3_2 = phase3_gen(2)
            phase2(3, filler=chain2(p3_1, p3_2), per_hp=2)
            drain(p3_1)
            drain(p3_2)
            emit_norm()
            drain(phase3_gen(3))

    nc.compile()
    return nc


def chain2(a, b):
    yield from a
    yield from b


def chain3(a, b, c):
    yield from a
    yield from b
    yield from c


def to_bf16(a: np.ndarray) -> np.ndarray:
    return np.ascontiguousarray(a, dtype=np.float32).astype(ml_dtypes.bfloat16)


def shard_inputs(x, W_qkv, W_out):
    """Build the 8 per-core input maps."""
    xT = [to_bf16(x[b].T) for b in range(B)]
    maps = []
    for core in range(NCORES):
        b, hf = core // 2, core % 2
        wq = W_qkv[:, hf * F : (hf + 1) * F]
        wk = W_qkv[:, C + hf * F : C + (hf + 1) * F]
        wv = W_qkv[:, 2 * C + hf * F : 2 * C + (hf + 1) * F]
        maps.append(
            {
                "xT": xT[b],
                "wqk": to_bf16(np.concatenate([wq, wk], axis=1)),
                "wv": to_bf16(wv),
                "wo": to_bf16(W_out[hf * F : (hf + 1) * F, :]),
            }
        )
    return maps


_NC_CACHE = {}


def get_nc():
    if "nc" not in _NC_CACHE:
        _NC_CACHE["nc"] = build_nc()
    return _NC_CACHE["nc"]


def kernel(x, W_qkv, W_out, _run_kwargs=None):
    x = np.asarray(x, dtype=np.float32)
    W_qkv = np.asarray(W_qkv, dtype=np.float32)
    W_out = np.asarray(W_out, dtype=np.float32)
    nc = get_nc()
    maps = shard_inputs(x, W_qkv, W_out)
    res = run_bass_kernel_spmd(nc, maps, list(range(NCORES)), **(_run_kwargs or {}))
    out = np.empty((B, T, C), dtype=np.float32)
    for b in range(B):
        yT0 = np.asarray(res.results[2 * b]["yT"], dtype=np.float32)
        yT1 = np.asarray(res.results[2 * b + 1]["yT"], dtype=np.float32)
        out[b] = (yT0 + yT1).T
    if _run_kwargs is not None:
        _NC_CACHE["last_results"] = res
    return out
